# revision 2
# baseline (speedup 1.0000x reference)
"""Mamba chunk-state kernel for Trainium2 (8 NeuronCores, Bass/Tile) — v2.

Computes, for inputs
    B  (b=4, s=8192, g=1, n=128)   f32
    x  (b=4, s=8192, h=32, p=64)   f32
    dt (b=4, h=32, c=32, l=256)    f32
    dA (b=4, h=32, c=32, l=256)    f32
the chunked state update
    states[b,c,h,p,n] = sum_l x[b,c,l,h,p] * scale[b,h,c,l] * B[b,c,l,n]
    scale = exp(dA[...,-1:] - dA) * dt

Sharding: core i handles batch b = i//2 and chunk range (i%2)*16..+16.
Each (b, chunk-range) slice is fully independent -> no collectives.

v2 design (HBM-roofline driven):
  - x and B are cast to bf16 and repacked on the host so every device DMA
    is a fully contiguous [128, big] block: x in 2-chunk superblocks
    [128, 4x2048], B resident as one [128, 32x128] tile. Output is
    written bf16 in [n, hp] (transposed) layout, one contiguous DMA per
    superblock; the host transposes/upcasts. Total HBM traffic per core
    drops from 53.5 MB (all-f32) to 27.2 MB.
  - Matmuls run transposed: states_T[n, hp] = sum_l B[l,n] * xw[l,hp],
    bf16 at 1 cycle/row, 8 matmuls of 512 moving cols per chunk.
  - scale is computed on-device as in v1 (ACT exp, DVE mult, PE
    transpose to [l,h]); the x scaling uses stride-0 broadcast APs:
    one DVE tensor_tensor per half plus a GPSIMD slice so DVE/Pool/ACT
    all stay under the DMA time.
"""

import numpy as np

BATCH, SEQLEN, NGROUPS, DSTATE = 4, 8192, 1, 128
NHEADS, HEADDIM, CHUNK = 32, 64, 256
NCHUNKS = SEQLEN // CHUNK  # 32
NCORES = 8
CPC = (BATCH * NCHUNKS) // NCORES  # 16 chunks per core
NSB = CPC // 2  # 8 two-chunk superblocks
HP = NHEADS * HEADDIM  # 2048

# heads of the second l-half scaled on GPSIMD (rest on DVE)
POOL_HEADS = 22

_cached_nc = None


def _build_nc(repeat=1):
    import concourse.bacc as bacc
    import concourse.mybir as mybir
    import concourse.tile as tile
    from concourse.masks import make_identity

    f32 = mybir.dt.float32
    bf16 = mybir.dt.bfloat16
    Exp = mybir.ActivationFunctionType.Exp

    nc = bacc.Bacc(
        "TRN2",
        target_bir_lowering=False,
        debug=False,
        num_devices=NCORES,
    )

    # x: [sb, l, t*2048] bf16, t = chunk_in_sb*2 + half (4 halfblocks/sb)
    x_d = nc.dram_tensor("x_s", [NSB, 128, 4 * HP], bf16, kind="ExternalInput").ap()
    # B: [l, t*128] bf16, t = chunk*2 + half (32 halfblocks)
    b_d = nc.dram_tensor("b_s", [128, 2 * CPC * DSTATE], bf16, kind="ExternalInput").ap()
    dt_d = nc.dram_tensor("dt_s", [NHEADS, CPC * CHUNK], f32, kind="ExternalInput").ap()
    da_d = nc.dram_tensor("da_s", [NHEADS, CPC * CHUNK], f32, kind="ExternalInput").ap()
    # out: [sb, n, c2*2048 + hp] bf16 (states transposed; host fixes layout)
    out_d = nc.dram_tensor("out_s", [NSB, DSTATE, 2 * HP], bf16, kind="ExternalOutput").ap()

    with tile.TileContext(nc) as tc:
        with (
            tc.tile_pool(name="const", bufs=1) as const_pool,
            tc.tile_pool(name="meta", bufs=1) as meta_pool,
            tc.tile_pool(name="xin", bufs=2) as x_pool,
            tc.tile_pool(name="xwp", bufs=4) as xw_pool,
            tc.tile_pool(name="scp", bufs=4) as sc_pool,
            tc.tile_pool(name="stgp", bufs=2) as stg_pool,
            tc.tile_pool(name="pstates", bufs=6, space="PSUM") as ps_pool,
            tc.tile_pool(name="ptrans", bufs=2, space="PSUM") as pt_pool,
        ):
            ident = const_pool.tile([32, 32], f32)
            make_identity(nc, ident)

            # resident per-core tensors
            dt_t = meta_pool.tile([NHEADS, CPC * CHUNK], f32)
            da_t = meta_pool.tile([NHEADS, CPC * CHUNK], f32)
            b_t = meta_pool.tile([128, 2 * CPC * DSTATE], bf16)
            nc.sync.dma_start(dt_t[:], dt_d[:])
            nc.sync.dma_start(da_t[:], da_d[:])
            nc.sync.dma_start(b_t[:], b_d[:])

            for sb_rep in range(NSB * repeat):
                sb = sb_rep % NSB
                # ---- superblock loads: one contiguous DMA each way ----
                xt = x_pool.tile([128, 4 * HP], bf16, name="xt", tag="xt")
                nc.sync.dma_start(xt[:], x_d[sb])
                stg = stg_pool.tile([DSTATE, 2 * HP], bf16, name="stg", tag="stg")

                for c2 in range(2):
                    cc = 2 * sb + c2
                    r0 = cc * CHUNK
                    # ---- scale = exp(dA_last - dA) * dt in [h, l] ----
                    dec = sc_pool.tile([NHEADS, CHUNK], f32, name="dec", tag="dec")
                    nc.scalar.activation(
                        dec[:],
                        da_t[:, r0 : r0 + CHUNK],
                        Exp,
                        bias=da_t[:, r0 + CHUNK - 1 : r0 + CHUNK],
                        scale=-1.0,
                    )
                    scl = sc_pool.tile([NHEADS, CHUNK], f32, name="scl", tag="scl")
                    nc.vector.tensor_mul(scl[:], dec[:], dt_t[:, r0 : r0 + CHUNK])

                    # ---- transpose scale to [l, h]: cols 0:32 half0, 32:64 half1
                    ptr = pt_pool.tile([128, 64], f32, name="ptr", tag="ptr")
                    nc.tensor.transpose(ptr[:, 0:32], scl[:, 0:128], ident[:])
                    nc.tensor.transpose(ptr[:, 32:64], scl[:, 128:256], ident[:])
                    sct = sc_pool.tile([128, 64], f32, name="sct", tag="sct")
                    nc.scalar.copy(sct[:], ptr[:])

                    # ---- xw = x * scale (broadcast over p), bf16 out ----
                    xw = xw_pool.tile([128, 2 * HP], bf16, name="xw", tag="xw")
                    xv = xt[:, c2 * 2 * HP : (c2 + 1) * 2 * HP]
                    # half 0: all 32 heads on DVE
                    nc.vector.tensor_tensor(
                        xw[:, 0:HP].rearrange("l (h p) -> l h p", h=NHEADS),
                        xv[:, 0:HP].rearrange("l (h p) -> l h p", h=NHEADS),
                        sct[:, 0:32].broadcast_to([128, 32, HEADDIM]),
                        mybir.AluOpType.mult,
                    )
                    # half 1: split DVE / GPSIMD
                    dh = NHEADS - POOL_HEADS
                    c0 = HP
                    nc.gpsimd.tensor_tensor(
                        xw[:, c0 : c0 + POOL_HEADS * HEADDIM].rearrange(
                            "l (h p) -> l h p", h=POOL_HEADS
                        ),
                        xv[:, c0 : c0 + POOL_HEADS * HEADDIM].rearrange(
                            "l (h p) -> l h p", h=POOL_HEADS
                        ),
                        sct[:, 32 : 32 + POOL_HEADS].broadcast_to(
                            [128, POOL_HEADS, HEADDIM]
                        ),
                        mybir.AluOpType.mult,
                    )
                    c1 = HP + POOL_HEADS * HEADDIM
                    nc.vector.tensor_tensor(
                        xw[:, c1 : c1 + dh * HEADDIM].rearrange(
                            "l (h p) -> l h p", h=dh
                        ),
                        xv[:, c1 : c1 + dh * HEADDIM].rearrange(
                            "l (h p) -> l h p", h=dh
                        ),
                        sct[:, 32 + POOL_HEADS : 64].broadcast_to(
                            [128, dh, HEADDIM]
                        ),
                        mybir.AluOpType.mult,
                    )

                    # ---- states_T[n, hp] = sum_l B[l,n] xw[l,hp] ----
                    bl0 = b_t[:, (2 * cc) * DSTATE : (2 * cc + 1) * DSTATE]
                    bl1 = b_t[:, (2 * cc + 1) * DSTATE : (2 * cc + 2) * DSTATE]
                    for q in range(4):
                        st = ps_pool.tile([DSTATE, 512], f32, name="st", tag="st")
                        nc.tensor.matmul(
                            st[:], bl0, xw[:, q * 512 : (q + 1) * 512],
                            start=True, stop=False,
                        )
                        nc.tensor.matmul(
                            st[:], bl1, xw[:, HP + q * 512 : HP + (q + 1) * 512],
                            start=False, stop=True,
                        )
                        nc.scalar.copy(
                            stg[:, c2 * HP + q * 512 : c2 * HP + (q + 1) * 512],
                            st[:],
                        )

                # one contiguous out DMA per superblock
                nc.sync.dma_start(out_d[sb], stg[:])

    nc.compile()
    return nc


def _get_nc():
    global _cached_nc
    if _cached_nc is None:
        _cached_nc = _build_nc()
    return _cached_nc


def _in_maps(B, x, dt, dA_cumsum):
    import ml_dtypes

    bf16 = ml_dtypes.bfloat16
    B = np.asarray(B, dtype=np.float32)
    x = np.asarray(x, dtype=np.float32)
    dt = np.asarray(dt, dtype=np.float32)
    dA = np.asarray(dA_cumsum, dtype=np.float32)
    maps = []
    for core in range(NCORES):
        b = core // 2
        c0 = (core % 2) * CPC
        s0, s1 = c0 * CHUNK, (c0 + CPC) * CHUNK
        # x: [4096, 2048] -> [sb, l, (c2 half), hp] -> [8, 128, 8192]
        xs = (
            x[b, s0:s1]
            .reshape(NSB, 2, 2, 128, HP)
            .transpose(0, 3, 1, 2, 4)
            .reshape(NSB, 128, 4 * HP)
            .astype(bf16)
        )
        # B: [4096, 128] -> [t, l, n] -> [l, t, n] -> [128, 4096]
        bs = (
            B[b, s0:s1, 0, :]
            .reshape(2 * CPC, 128, DSTATE)
            .transpose(1, 0, 2)
            .reshape(128, 2 * CPC * DSTATE)
            .astype(bf16)
        )
        maps.append(
            {
                "x_s": np.ascontiguousarray(xs),
                "b_s": np.ascontiguousarray(bs),
                "dt_s": np.ascontiguousarray(
                    dt[b, :, c0 : c0 + CPC, :]
                ).reshape(NHEADS, CPC * CHUNK),
                "da_s": np.ascontiguousarray(
                    dA[b, :, c0 : c0 + CPC, :]
                ).reshape(NHEADS, CPC * CHUNK),
            }
        )
    return maps


def _assemble(results):
    out = np.empty((BATCH, NCHUNKS, NHEADS, HEADDIM, DSTATE), np.float32)
    for core in range(NCORES):
        b = core // 2
        c0 = (core % 2) * CPC
        o = np.asarray(results[core]["out_s"])  # [8, 128, 4096] bf16
        o = (
            o.astype(np.float32)
            .reshape(NSB, DSTATE, 2, NHEADS, HEADDIM)
            .transpose(0, 2, 3, 4, 1)
            .reshape(CPC, NHEADS, HEADDIM, DSTATE)
        )
        out[b, c0 : c0 + CPC] = o
    return out


def _run(B, x, dt, dA_cumsum, **run_kwargs):
    from concourse import bass_utils

    nc = _get_nc()
    res = bass_utils.run_bass_kernel_spmd(
        nc, _in_maps(B, x, dt, dA_cumsum), core_ids=list(range(NCORES)), **run_kwargs
    )
    return _assemble(res.results), res


def kernel(B, x, dt, dA_cumsum):
    out, _ = _run(B, x, dt, dA_cumsum)
    return out


# revision 16
# speedup vs baseline: 2.6165x; 2.6165x over previous
"""Mamba chunk-state kernel for Trainium2 (8 NeuronCores, Bass/Tile) — v4.

Computes, for inputs
    B  (b=4, s=8192, g=1, n=128)   f32
    x  (b=4, s=8192, h=32, p=64)   f32
    dt (b=4, h=32, c=32, l=256)    f32
    dA (b=4, h=32, c=32, l=256)    f32
the chunked state update
    states[b,c,h,p,n] = sum_l x[b,c,l,h,p] * scale[b,h,c,l] * B[b,c,l,n]
    scale = exp(dA[...,-1:] - dA) * dt

Sharding: core i handles batch b = i//2 and chunk range (i%2)*16..+16.
Each (b, chunk-range) slice is fully independent -> no collectives.

Design (HBM-roofline driven; ~27 MB/core of traffic vs 53.5 all-f32):
  - x and B are cast to bf16 and repacked on the host so every device DMA
    is a fully contiguous [128, big] block: x in 4-chunk superblocks
    (one [128, 32KB] DMA each), B resident as one [128, 8KB] tile.
    Output is written bf16 in [n, hp] (transposed) layout, one contiguous
    [128, 32KB] DMA per 8 chunks; the host transposes/upcasts.
  - Matmuls run transposed: states_T[n, hp] = sum_l B[l,n] * xw[l,hp],
    bf16 at 1 cycle/row, 8 matmuls of 512 moving cols per chunk,
    single-bank PSUM tiles evacuated by ACT (f32 -> bf16).
  - Phase 1 of each pass precomputes all 16 chunks' scales, transposed
    to [l, h] via PE, into one resident sct_all tile, so the per-chunk
    x -> xw -> matmul -> evac -> store pipeline has no cross-engine
    scale chain on its critical path. The x scaling is one DVE
    tensor_tensor per l-half with a stride-0 broadcast scale AP.
"""

import numpy as np

BATCH, SEQLEN, NGROUPS, DSTATE = 4, 8192, 1, 128
NHEADS, HEADDIM, CHUNK = 32, 64, 256
NCHUNKS = SEQLEN // CHUNK  # 32
NCORES = 8
CPC = (BATCH * NCHUNKS) // NCORES  # 16 chunks per core
NSB = CPC // 4  # 4 four-chunk superblocks
HP = NHEADS * HEADDIM  # 2048

_cached_nc = None


def _build_nc(repeat=1):
    import concourse.bacc as bacc
    import concourse.mybir as mybir
    import concourse.tile as tile
    from concourse.masks import make_identity

    f32 = mybir.dt.float32
    bf16 = mybir.dt.bfloat16
    Exp = mybir.ActivationFunctionType.Exp

    nc = bacc.Bacc(
        "TRN2",
        target_bir_lowering=False,
        debug=False,
        num_devices=NCORES,
    )

    # x: [sb, l, t*2048] bf16, t = chunk_in_sb*2 + half (8 halfblocks/sb)
    x_d = nc.dram_tensor("x_s", [NSB, 128, 8 * HP], bf16, kind="ExternalInput").ap()
    # B: [l, t*128] bf16, t = chunk*2 + half (32 halfblocks)
    b_d = nc.dram_tensor("b_s", [128, 2 * CPC * DSTATE], bf16, kind="ExternalInput").ap()
    dt_d = nc.dram_tensor("dt_s", [NHEADS, CPC * CHUNK], f32, kind="ExternalInput").ap()
    da_d = nc.dram_tensor("da_s", [NHEADS, CPC * CHUNK], f32, kind="ExternalInput").ap()
    # out: [half, n, cc*2048 + hp] bf16 (states transposed; host fixes layout)
    out_d = nc.dram_tensor("out_s", [2, DSTATE, 8 * HP], bf16, kind="ExternalOutput").ap()

    with tile.TileContext(nc) as tc:
        with (
            tc.tile_pool(name="const", bufs=1) as const_pool,
            tc.tile_pool(name="meta", bufs=1) as meta_pool,
            tc.tile_pool(name="xin", bufs=2) as x_pool,
            tc.tile_pool(name="xwp", bufs=3) as xw_pool,
            tc.tile_pool(name="scp", bufs=3) as sc_pool,
            tc.tile_pool(name="sctp", bufs=2) as sct_pool,
            tc.tile_pool(name="stgp", bufs=2) as stg_pool,
            tc.tile_pool(name="pstates", bufs=6, space="PSUM") as ps_pool,
            tc.tile_pool(name="ptrans", bufs=2, space="PSUM") as pt_pool,
        ):
            ident = const_pool.tile([32, 32], f32)
            make_identity(nc, ident)

            # resident per-core tensors
            dt_t = meta_pool.tile([NHEADS, CPC * CHUNK], f32)
            da_t = meta_pool.tile([NHEADS, CPC * CHUNK], f32)
            b_t = meta_pool.tile([128, 2 * CPC * DSTATE], bf16)
            nc.sync.dma_start(dt_t[:], dt_d[:])
            nc.sync.dma_start(da_t[:], da_d[:])
            nc.sync.dma_start(b_t[:], b_d[:])

            for _rep in range(repeat):
                # ---- phase 1: scales for all chunks -> sct_all [l, cc*64+th]
                # cols cc*64+0:32 = heads for l-half 0, +32:64 = l-half 1
                sct_all = sct_pool.tile([128, CPC * 64], f32, name="sct_all", tag="sa")
                for g in range(4):  # 4 chunks per PSUM bank
                    ptr = pt_pool.tile([128, 256], f32, name="ptr", tag="ptr")
                    for k in range(4):
                        cc = g * 4 + k
                        r0 = cc * CHUNK
                        dec = sc_pool.tile([NHEADS, CHUNK], f32, name="dec", tag="dec")
                        nc.scalar.activation(
                            dec[:],
                            da_t[:, r0 : r0 + CHUNK],
                            Exp,
                            bias=da_t[:, r0 + CHUNK - 1 : r0 + CHUNK],
                            scale=-1.0,
                        )
                        scl = sc_pool.tile([NHEADS, CHUNK], f32, name="scl", tag="scl")
                        nc.vector.tensor_mul(scl[:], dec[:], dt_t[:, r0 : r0 + CHUNK])
                        nc.tensor.transpose(
                            ptr[:, k * 64 : k * 64 + 32], scl[:, 0:128], ident[:]
                        )
                        nc.tensor.transpose(
                            ptr[:, k * 64 + 32 : k * 64 + 64], scl[:, 128:256], ident[:]
                        )
                    nc.scalar.copy(sct_all[:, g * 256 : (g + 1) * 256], ptr[:])

                # ---- phase 2: superblock pipeline ----
                for sb in range(NSB):
                    xt = x_pool.tile([128, 8 * HP], bf16, name="xt", tag="xt")
                    nc.sync.dma_start(xt[:], x_d[sb])
                    if sb % 2 == 0:
                        stg = stg_pool.tile([DSTATE, 8 * HP], bf16, name="stg", tag="stg")

                    for c2 in range(4):
                        cc = 4 * sb + c2
                        xv = xt[:, c2 * 2 * HP : (c2 + 1) * 2 * HP]
                        s0 = cc * 64

                        # xw = x * scale (broadcast over p), bf16 out
                        xw = xw_pool.tile([128, 2 * HP], bf16, name="xw", tag="xw")
                        nc.vector.tensor_tensor(
                            xw[:, 0:HP].rearrange("l (h p) -> l h p", h=NHEADS),
                            xv[:, 0:HP].rearrange("l (h p) -> l h p", h=NHEADS),
                            sct_all[:, s0 : s0 + 32].broadcast_to(
                                [128, NHEADS, HEADDIM]
                            ),
                            mybir.AluOpType.mult,
                        )
                        nc.vector.tensor_tensor(
                            xw[:, HP : 2 * HP].rearrange("l (h p) -> l h p", h=NHEADS),
                            xv[:, HP : 2 * HP].rearrange("l (h p) -> l h p", h=NHEADS),
                            sct_all[:, s0 + 32 : s0 + 64].broadcast_to(
                                [128, NHEADS, HEADDIM]
                            ),
                            mybir.AluOpType.mult,
                        )

                        # states_T[n, hp] = sum_l B[l,n] xw[l,hp]
                        bl0 = b_t[:, (2 * cc) * DSTATE : (2 * cc + 1) * DSTATE]
                        bl1 = b_t[:, (2 * cc + 1) * DSTATE : (2 * cc + 2) * DSTATE]
                        so = ((sb % 2) * 4 + c2) * HP
                        for q in range(4):
                            st = ps_pool.tile([DSTATE, 512], f32, name="st", tag="st")
                            nc.tensor.matmul(
                                st[:], bl0, xw[:, q * 512 : (q + 1) * 512],
                                start=True, stop=False,
                            )
                            nc.tensor.matmul(
                                st[:], bl1, xw[:, HP + q * 512 : HP + (q + 1) * 512],
                                start=False, stop=True,
                            )
                            nc.scalar.copy(
                                stg[:, so + q * 512 : so + (q + 1) * 512], st[:]
                            )

                    # one contiguous out DMA per 2 superblocks (8 chunks)
                    if sb % 2 == 1:
                        nc.sync.dma_start(out_d[sb // 2], stg[:])

    nc.compile()
    return nc


def _get_nc():
    global _cached_nc
    if _cached_nc is None:
        _cached_nc = _build_nc()
    return _cached_nc


def _in_maps(B, x, dt, dA_cumsum):
    import ml_dtypes

    bf16 = ml_dtypes.bfloat16
    B = np.asarray(B, dtype=np.float32)
    x = np.asarray(x, dtype=np.float32)
    dt = np.asarray(dt, dtype=np.float32)
    dA = np.asarray(dA_cumsum, dtype=np.float32)
    maps = []
    for core in range(NCORES):
        b = core // 2
        c0 = (core % 2) * CPC
        s0, s1 = c0 * CHUNK, (c0 + CPC) * CHUNK
        # x: [4096, 2048] -> [sb, l, (c4 half), hp] -> [4, 128, 16384]
        xs = (
            x[b, s0:s1]
            .reshape(NSB, 4, 2, 128, HP)
            .transpose(0, 3, 1, 2, 4)
            .reshape(NSB, 128, 8 * HP)
            .astype(bf16)
        )
        # B: [4096, 128] -> [t, l, n] -> [l, t, n] -> [128, 4096]
        bs = (
            B[b, s0:s1, 0, :]
            .reshape(2 * CPC, 128, DSTATE)
            .transpose(1, 0, 2)
            .reshape(128, 2 * CPC * DSTATE)
            .astype(bf16)
        )
        maps.append(
            {
                "x_s": np.ascontiguousarray(xs),
                "b_s": np.ascontiguousarray(bs),
                "dt_s": np.ascontiguousarray(
                    dt[b, :, c0 : c0 + CPC, :]
                ).reshape(NHEADS, CPC * CHUNK),
                "da_s": np.ascontiguousarray(
                    dA[b, :, c0 : c0 + CPC, :]
                ).reshape(NHEADS, CPC * CHUNK),
            }
        )
    return maps


def _assemble(results):
    out = np.empty((BATCH, NCHUNKS, NHEADS, HEADDIM, DSTATE), np.float32)
    for core in range(NCORES):
        b = core // 2
        c0 = (core % 2) * CPC
        o = np.asarray(results[core]["out_s"])  # [2, 128, 16384] bf16
        o = (
            o.astype(np.float32)
            .reshape(2, DSTATE, 8, NHEADS, HEADDIM)
            .transpose(0, 2, 3, 4, 1)
            .reshape(CPC, NHEADS, HEADDIM, DSTATE)
        )
        out[b, c0 : c0 + CPC] = o
    return out


def _run(B, x, dt, dA_cumsum, **run_kwargs):
    from concourse import bass_utils

    nc = _get_nc()
    res = bass_utils.run_bass_kernel_spmd(
        nc, _in_maps(B, x, dt, dA_cumsum), core_ids=list(range(NCORES)), **run_kwargs
    )
    return _assemble(res.results), res


def kernel(B, x, dt, dA_cumsum):
    out, _ = _run(B, x, dt, dA_cumsum)
    return out
